# revision 27
# baseline (speedup 1.0000x reference)
"""Trainium2 Bass kernel for GQA attention (B=2, S=2048, D=2048, 16 q-heads,
4 kv-heads, head_dim=128, RoPE, causal) sharded over 8 NeuronCores.

Sharding: core c handles batch b = c//4 and q-head group g = c%4
(q-heads 4g..4g+3, which share kv-head g).  Each core computes a partial
output o_part[b] = sum_{its heads} attn_head @ Wo_head; the host sums the
4 partials per batch.

RoPE cos/sin tables are precomputed on the host from `positions` (host
preprocessing, same class as the host x-transpose / bf16 casts) and DMA'd
in as parameters, so the device spends no time on the sin/cos chain.
"""

import sys

sys.path.insert(0, "/opt/trn_rl_repo")

import math

import ml_dtypes
import numpy as np

P = 128
NEG = -1.0e9
EXP_BIAS = -8.0  # exp(s - 8): cancels in softmax normalization, avoids overflow


def build_nc(S=2048, D=2048, QH=4, H=128, theta=10000.0):
    """Build the per-core Bass graph.

    Per-core problem: xt [D, S] bf16, cos2/sin2s [P, S] f32,
    wq [QH, D, H] bf16 (pre-scaled by 1/sqrt(H)), wk/wv [D, H] bf16,
    wo [QH, H, D] bf16  ->  o [S, D] f32 (partial over heads).
    """
    import concourse.bacc as bacc
    import concourse.mybir as mybir
    from concourse import tile
    from concourse.masks import make_identity

    f32 = mybir.dt.float32
    bf16 = mybir.dt.bfloat16
    ADD = mybir.AluOpType.add
    MULT = mybir.AluOpType.mult
    EXP = mybir.ActivationFunctionType.Exp

    assert H == P
    HH = H // 2  # 64
    DK = D // P  # 16 d-chunks
    SB = min(512, S)  # sq block width
    NSB = S // SB  # 4 sq blocks (= quarters)
    RB = SB // P  # 4 sq subtiles per block
    NT = S // P  # 16 t tiles
    NSQ = S // P

    nc = bacc.Bacc(None, target_bir_lowering=False)

    xq_d = nc.declare_dram_parameter("xq", [S // 512, P, 4 * D], bf16, isOutput=False)
    cos_d = nc.declare_dram_parameter("cos2", [P, S], bf16, isOutput=False)
    sin_d = nc.declare_dram_parameter("sin2s", [P, S], bf16, isOutput=False)
    wq_d = nc.declare_dram_parameter("wq", [QH, D, H], bf16, isOutput=False)
    wk_d = nc.declare_dram_parameter("wk", [D, H], bf16, isOutput=False)
    wv_d = nc.declare_dram_parameter("wv", [D, H], bf16, isOutput=False)
    wo_d = nc.declare_dram_parameter("wo", [QH, H, D], bf16, isOutput=False)
    o_d = nc.declare_dram_parameter("o", [S, D], f32, isOutput=True)

    from contextlib import ExitStack

    with tile.TileContext(nc) as tc, ExitStack() as es:
        # ---------------- pools ----------------
        const = es.enter_context(tc.tile_pool(name="const", bufs=1))
        persist = es.enter_context(tc.tile_pool(name="persist", bufs=1))
        small = es.enter_context(tc.tile_pool(name="small", bufs=2))
        pt_pool = es.enter_context(tc.tile_pool(name="pt", bufs=8))
        ob_pool = es.enter_context(tc.tile_pool(name="ob", bufs=4))
        at_pool = es.enter_context(tc.tile_pool(name="at", bufs=1))
        # PSUM (8 banks): scores ring 2 + shared ring 2 (proj pq/pv,
        # o-proj po, attn transpose ptr2) + 4 AV accumulator banks.
        ps_scr = es.enter_context(tc.tile_pool(name="ps_scr", bufs=2, space="PSUM"))
        ps_pp = es.enter_context(tc.tile_pool(name="ps_pp", bufs=2, space="PSUM"))
        ps_av = es.enter_context(tc.tile_pool(name="ps_av", bufs=1, space="PSUM"))

        # ---------------- constants ----------------
        ident = const.tile([P, P], bf16)
        make_identity(nc, ident)

        exp_bias = const.tile([P, 1], f32)
        nc.gpsimd.memset(exp_bias[:], EXP_BIAS)



        # ---------------- x load -----------------------------------------
        # xT layout [p, st, dk*P + u] : element x(s = st*P + u, d = dk*P + p).
        # The host pre-permutes x into this exact layout, quarter-major, so
        # each st-quarter is ONE fully contiguous DMA (16KB/partition).
        # Quarters 0/1 are split across the sync + scalar queues (scalar is
        # idle until attention starts at ~20us).
        xT = persist.tile([P, NSQ, D], bf16)

        def x_quarter(q, engines):
            n = len(engines)
            hw = RB // n
            for i, eng in enumerate(engines):
                eng.dma_start(
                    xT[:, q * RB + i * hw : q * RB + (i + 1) * hw, :],
                    xq_d[q, :, i * hw * D : (i + 1) * hw * D].rearrange(
                        "p (st d) -> p st d", d=D
                    ),
                )

        x_quarter(0, [nc.sync, nc.scalar])
        # wq1-3 ride the sync queue between x quarters 0 and 1: they are
        # needed at ~20/24/27us, before x q1 (~39us).  Quarters 2/3 are
        # emitted later, behind o-store DMAs, so their transfers don't
        # steal bandwidth from the loads that gate the pipeline start.

        # ---------------- weights (first-use order across queues) ---------
        def load_w_dh(dram_ap, name, eng):  # dram bf16 [D, H] -> sbuf [P, DK, H]
            wb = persist.tile([P, DK, H], bf16, name=name, tag=name)
            eng.dma_start(wb[:], dram_ap.rearrange("(k p) h -> p k h", p=P))
            return wb

        wq_sb = [load_w_dh(wq_d[0], "wq0", nc.gpsimd)]

        # rope tables (host-computed, bf16) right after wq0: first use is
        # the first block's rope-apply at ~15us.
        cos2 = persist.tile([P, S], bf16)
        sin2s = persist.tile([P, S], bf16)
        nc.gpsimd.dma_start(cos2[:], cos_d[:, :])
        nc.gpsimd.dma_start(sin2s[:], sin_d[:, :])

        wq_sb.append(load_w_dh(wq_d[1], "wq1", nc.scalar))
        for h in range(2, QH):
            wq_sb.append(load_w_dh(wq_d[h], f"wq{h}", nc.sync))
        wk_sb = load_w_dh(wk_d, "wk", nc.gpsimd)
        wv_sb = load_w_dh(wv_d, "wv", nc.gpsimd)
        x_quarter(1, [nc.sync])

        # wo tiles declared here; DMAs emitted on gpsimd just before the
        # first attention block (needed at ~40us, keeps them out of the
        # early bandwidth wave).
        wo_sb = []
        for h in range(QH):
            wb = persist.tile([P, D], bf16, name=f"wo{h}", tag=f"wo{h}")
            wo_sb.append(wb)

        def load_wo():
            for h in range(QH):
                nc.gpsimd.dma_start(wo_sb[h][:], wo_d[h])

        # ---------------- q/k projection block (with rope) ----------------
        def proj_qk(w_sb, out_tile, sb):
            pq = ps_pp.tile([P, SB], f32, tag="pp", name="pq")
            for dk in range(DK):
                nc.tensor.matmul(
                    pq[:],
                    w_sb[:, dk, :],
                    xT[:, sb * RB : (sb + 1) * RB, dk * P : (dk + 1) * P],
                    start=(dk == 0),
                    stop=(dk == DK - 1),
                )
            sl = slice(sb * SB, (sb + 1) * SB)
            # rope: out = pq * cos2 + rot(pq) * sin2s
            tsin = small.tile([P, SB], f32, tag="tsin")
            nc.vector.tensor_tensor(
                tsin[0:HH, :], pq[HH:P, :], sin2s[0:HH, sl], MULT
            )
            nc.vector.tensor_tensor(
                tsin[HH:P, :], pq[0:HH, :], sin2s[HH:P, sl], MULT
            )
            tcos = small.tile([P, SB], f32, tag="tcos")
            nc.vector.tensor_tensor(tcos[:], pq[:], cos2[:, sl], MULT)
            nc.vector.tensor_tensor(out_tile[:, sl], tcos[:], tsin[:], ADD)

        qT = [
            persist.tile([P, S], bf16, name=f"qT{h}", tag=f"qT{h}")
            for h in range(QH)
        ]
        kT = persist.tile([P, S], bf16)

        # ---------------- v projection tile (v' with ones column) ---------
        # layout [P, NT, H+4]: v[t = tt*P + p, 0:H], v'[t, H] = 1
        VW = H + 4
        vp = persist.tile([P, NT, VW], bf16)

        def proj_v(tt):
            pv = ps_pp.tile([P, SB], f32, tag="pp", name="pv")[:, 0:P]
            for dk in range(DK):
                nc.tensor.matmul(
                    pv[:],
                    xT[:, tt, dk * P : (dk + 1) * P],
                    wv_sb[:, dk, :],
                    start=(dk == 0),
                    stop=(dk == DK - 1),
                )
            nc.vector.tensor_copy(vp[:, tt, 0:H], pv[:])
            nc.gpsimd.memset(vp[:, tt, H : H + 1], 1.0)

        # ---------------- attention + fused O projection ------------------
        # O-projection matmuls of block sb-1 are fed INTO the attention
        # groups of block sb: each group then carries ~650-900ns of PE work
        # vs the ~570ns scalar-engine exp period, so exp latency stays off
        # the critical path.
        class OprojFeeder:
            """PE filler work fed into attention groups: o-projection tiles
            of the previous block, then v-projection tiles of the next
            quarter (at the tail, after their x quarter has landed)."""

            def __init__(self, attnT_blk, sb, final=False):
                self.units = []  # one unit = one complete po / pv tile
                if attnT_blk is None:
                    self.total = 0
                    self.emitted = 0
                    return
                store_engs = [nc.sync, nc.scalar] if final else [nc.sync]

                def po_tile(st, db):
                    po = ps_pp.tile([P, SB], f32, tag="pp", name="po")
                    for h in range(QH):
                        nc.tensor.matmul(
                            po[:],
                            attnT_blk[h][
                                :, (st - RB * sb) * P : (st - RB * sb + 1) * P
                            ],
                            wo_sb[h][:, db * SB : (db + 1) * SB],
                            start=(h == 0),
                            stop=(h == QH - 1),
                        )
                    ob = ob_pool.tile([P, SB], f32, tag="ob")
                    nc.vector.tensor_copy(ob[:], po[:])
                    store_engs[(st * 4 + db) % len(store_engs)].dma_start(
                        o_d[st * P : (st + 1) * P, db * SB : (db + 1) * SB],
                        ob[:],
                    )

                for r2 in range(RB):
                    st = RB * sb + r2
                    for db in range(D // SB):
                        self.units.append(
                            lambda st=st, db=db: po_tile(st, db)
                        )
                self.total = len(self.units)
                self.emitted = 0

            def add_units(self, units, front=False):
                if front:
                    self.units = units + self.units
                else:
                    self.units.extend(units)
                self.total += len(units)

            def step(self, group_idx, groups_total):
                # Bresenham pacing: unit k fires at group k*T/N so the
                # feed stays even across the whole block.
                if not self.units:
                    return
                n = self.total
                while self.units and (
                    self.emitted * groups_total <= group_idx * n
                ):
                    self.units.pop(0)()
                    self.emitted += 1

            def drain(self):
                for u in self.units:
                    u()
                self.units = []

        def make_attention_block(sb, attnT, feeder, groups):
            """All heads of block sb as one flat group stream (the LOOK-2
            scores/exp pipeline runs across head boundaries).  Returns
            (warmup, run): warmup emits the first LOOK groups' scores and
            can be called between projection blocks so their exps cook on
            the scalar engine while the PE is still projecting."""
            ntt = RB * (sb + 1)
            LOOK = 2
            seq = [(h, tt) for h in range(QH) for tt in range(ntt)]
            pav_of = {}
            ans = {}
            pts = {}

            def emit_scores(h, tt):
                if (h, tt) in pts:
                    return
                r = tt - RB * sb
                c0 = max(0, r) * P
                pscore = ps_scr.tile([P, SB], f32, tag="scr", name="pscore")
                nc.tensor.matmul(
                    pscore[:, c0:SB],
                    kT[:, tt * P : (tt + 1) * P],
                    qT[h][:, sb * SB + c0 : (sb + 1) * SB],
                    start=True,
                    stop=True,
                )
                pt = pt_pool.tile([P, SB], bf16, tag="pt")
                nc.scalar.activation(
                    pt[:, c0:SB], pscore[:, c0:SB], EXP, bias=exp_bias[:]
                )
                if r >= 0:
                    # causal mask: zero the upper triangle of the diagonal
                    # subtile on gpsimd (idle queue; keeps the DVE out of
                    # the scores->exp->AV chain).
                    nc.gpsimd.affine_select(
                        out=pt[:, c0 : c0 + P],
                        in_=pt[:, c0 : c0 + P],
                        compare_op=mybir.AluOpType.is_ge,
                        fill=0.0,
                        base=0,
                        pattern=[[1, P]],
                        channel_multiplier=-1,
                    )
                pts[(h, tt)] = pt

            def finish_subtile(h, r2):
                rec = small.tile([P, 1], f32, tag="rec", bufs=4)
                nc.vector.reciprocal(rec[:], pav_of[h][r2][:, H : H + 1])
                an = small.tile([P, H], bf16, tag="an", bufs=4)
                nc.vector.tensor_scalar_mul(
                    an[:], pav_of[h][r2][:, 0:H], rec[:]
                )
                ans[(h, r2)] = an

            def emit_transpose(h, r2):
                ptr2 = ps_pp.tile([P, SB], bf16, tag="pp", name="ptr2")[:, 0:P]
                nc.tensor.transpose(ptr2, ans.pop((h, r2))[:], ident[:])
                nc.vector.tensor_copy(
                    attnT[h][:, r2 * P : (r2 + 1) * P], ptr2
                )

            def warmup():
                for i in range(min(LOOK, len(seq))):
                    emit_scores(*seq[i])

            def run():
                warmup()
                feeder.step(groups[0], groups[1])
                groups[0] += 1
                for i, (h, tt) in enumerate(seq):
                    if tt == 0:
                        pav_of[h] = [
                            ps_av.tile(
                                [P, H + 1], f32, name=f"pav{r}",
                                tag=f"av{r}", bufs=1,
                            )[:]
                            for r in range(RB)
                        ]
                    r = tt - RB * sb
                    pt = pts.pop((h, tt))
                    for r2 in range(max(0, r), RB):
                        q128 = RB * sb + r2
                        nc.tensor.matmul(
                            pav_of[h][r2],
                            pt[:, r2 * P : (r2 + 1) * P],
                            vp[:, tt, 0 : H + 1],
                            start=(tt == 0),
                            stop=(tt == q128),
                        )
                    if r >= 0:
                        finish_subtile(h, r)
                    if r >= 1:
                        emit_transpose(h, r - 1)
                    if tt == ntt - 1:
                        emit_transpose(h, RB - 1)
                    feeder.step(groups[0], groups[1])
                    groups[0] += 1
                    if i + LOOK < len(seq):
                        emit_scores(*seq[i + LOOK])

            return warmup, run

        # ---------------- main schedule: per quarter ----------------------
        prev = None
        for sb in range(NSB):
            proj_qk(wq_sb[0], qT[0], sb)
            proj_qk(wk_sb, kT, sb)
            if sb == 0:
                for tt in range(0, RB):
                    proj_v(tt)
            attnT = [
                at_pool.tile(
                    [P, SB], bf16, name=f"attnT{h}", tag=f"attnT{h}", bufs=2
                )
                for h in range(QH)
            ]
            feeder = OprojFeeder(prev, sb - 1)
            if sb + 1 < NSB:
                feeder.add_units(
                    [lambda tt=tt: proj_v(tt)
                     for tt in range((sb + 1) * RB, (sb + 2) * RB)]
                )
            groups_left = [0, QH * RB * (sb + 1) + 1]
            if sb == 0:
                load_wo()
            if sb in (1, 2):
                x_quarter(sb + 1, [nc.sync])
            warmup, run = make_attention_block(sb, attnT, feeder, groups_left)
            # first attention scores between the projection blocks: their
            # exps cook on the scalar engine while the PE projects h1-3.
            warmup()
            for h in range(1, QH):
                proj_qk(wq_sb[h], qT[h], sb)
            run()
            feeder.drain()
            prev = attnT
        OprojFeeder(prev, NSB - 1, final=True).drain()

    nc.compile()
    return nc


_NC_CACHE = {}


def _get_nc(key):
    if key not in _NC_CACHE:
        _NC_CACHE[key] = build_nc(*key)
    return _NC_CACHE[key]


def _rope_tables(positions, H=128, theta=10000.0):
    """cos2/sin2s [P, S] f32 tables matching the device layout:
    cos2[p, s] = cos(ang[s, p mod HH]); sin2s[p, s] = -sin for p < HH,
    +sin for p >= HH."""
    HH = H // 2
    inv_ts = theta ** (-2.0 * np.arange(HH, dtype=np.float64) / H)
    ang = positions.astype(np.float64)[None, :] * inv_ts[:, None]  # [HH, S]
    cos = np.cos(ang).astype(np.float32)
    sin = np.sin(ang).astype(np.float32)
    cos2 = np.concatenate([cos, cos], axis=0).astype(ml_dtypes.bfloat16)
    sin2s = np.concatenate([-sin, sin], axis=0).astype(ml_dtypes.bfloat16)
    return np.ascontiguousarray(cos2), np.ascontiguousarray(sin2s)


def make_in_maps(x, positions, Wq, Wk, Wv, Wo, n_cores=8):
    B, S, D = x.shape
    Q, _, H = Wq.shape
    N = Wk.shape[0]
    groups = Q // N if N else 1
    gpb = n_cores // B  # head groups per batch (4)
    qh_per_core = Q // gpb
    assert qh_per_core * gpb == Q
    scale = np.float32(1.0 / math.sqrt(H))
    cos2, sin2s = _rope_tables(positions, H)
    # xq[q][p, stl, dk, u] = x[(q*4+stl)*128 + u, dk*128 + p]: the exact
    # on-chip xT layout, quarter-major, so each quarter is one plain DMA.
    DKh, RBh, NQh = D // P, 4, S // 512
    xq_b = [
        np.ascontiguousarray(
            x[b]
            .astype(ml_dtypes.bfloat16)
            .reshape(NQh, RBh, P, DKh, P)
            .transpose(0, 4, 1, 3, 2)
            .reshape(NQh, P, RBh * D)
        )
        for b in range(B)
    ]
    in_maps = []
    for c in range(n_cores):
        b = c // gpb
        g = c % gpb
        qh0 = g * qh_per_core
        kvh = qh0 // groups
        in_maps.append(
            {
                "xq": xq_b[b],
                "cos2": cos2,
                "sin2s": sin2s,
                "wq": np.ascontiguousarray(
                    (Wq[qh0 : qh0 + qh_per_core] * scale).astype(ml_dtypes.bfloat16)
                ),
                "wk": np.ascontiguousarray(Wk[kvh].astype(ml_dtypes.bfloat16)),
                "wv": np.ascontiguousarray(Wv[kvh].astype(ml_dtypes.bfloat16)),
                "wo": np.ascontiguousarray(
                    Wo[qh0 : qh0 + qh_per_core].astype(ml_dtypes.bfloat16)
                ),
            }
        )
    return in_maps, gpb, qh_per_core


def kernel(x, positions, Wq, Wk, Wv, Wo):
    """Full inputs -> full output.  x [B,S,D] f32, positions [S] i32,
    Wq [Q,D,H], Wk/Wv [N,D,H], Wo [Q,H,D].  Returns [B,S,D] f32."""
    from concourse.bass_utils import run_bass_kernel_spmd

    x = np.ascontiguousarray(np.asarray(x, dtype=np.float32))
    positions = np.ascontiguousarray(np.asarray(positions, dtype=np.int32))
    Wq = np.asarray(Wq, dtype=np.float32)
    Wk = np.asarray(Wk, dtype=np.float32)
    Wv = np.asarray(Wv, dtype=np.float32)
    Wo = np.asarray(Wo, dtype=np.float32)

    B, S, D = x.shape
    Q, _, H = Wq.shape
    n_cores = 8
    in_maps, gpb, qh_per_core = make_in_maps(x, positions, Wq, Wk, Wv, Wo, n_cores)

    nc = _get_nc((S, D, qh_per_core, H))
    res = run_bass_kernel_spmd(nc, in_maps, core_ids=list(range(n_cores)))
    out = np.zeros((B, S, D), dtype=np.float32)
    for c in range(n_cores):
        out[c // gpb] += res.results[c]["o"]
    return out


# revision 28
# speedup vs baseline: 1.0069x; 1.0069x over previous
"""Trainium2 Bass kernel for GQA attention (B=2, S=2048, D=2048, 16 q-heads,
4 kv-heads, head_dim=128, RoPE, causal) sharded over 8 NeuronCores.

Sharding: core c handles batch b = c//4 and q-head group g = c%4
(q-heads 4g..4g+3, which share kv-head g).  Each core computes a partial
output o_part[b] = sum_{its heads} attn_head @ Wo_head; the host sums the
4 partials per batch.

RoPE cos/sin tables are precomputed on the host from `positions` (host
preprocessing, same class as the host x-transpose / bf16 casts) and DMA'd
in as parameters, so the device spends no time on the sin/cos chain.
"""

import sys

sys.path.insert(0, "/opt/trn_rl_repo")

import math

import ml_dtypes
import numpy as np

P = 128
NEG = -1.0e9
EXP_BIAS = -8.0  # exp(s - 8): cancels in softmax normalization, avoids overflow


def build_nc(S=2048, D=2048, QH=4, H=128, theta=10000.0):
    """Build the per-core Bass graph.

    Per-core problem: xt [D, S] bf16, cos2/sin2s [P, S] f32,
    wq [QH, D, H] bf16 (pre-scaled by 1/sqrt(H)), wk/wv [D, H] bf16,
    wo [QH, H, D] bf16  ->  o [S, D] f32 (partial over heads).
    """
    import concourse.bacc as bacc
    import concourse.mybir as mybir
    from concourse import tile
    from concourse.masks import make_identity

    f32 = mybir.dt.float32
    bf16 = mybir.dt.bfloat16
    ADD = mybir.AluOpType.add
    MULT = mybir.AluOpType.mult
    EXP = mybir.ActivationFunctionType.Exp

    assert H == P
    HH = H // 2  # 64
    DK = D // P  # 16 d-chunks
    SB = min(512, S)  # sq block width
    NSB = S // SB  # 4 sq blocks (= quarters)
    RB = SB // P  # 4 sq subtiles per block
    NT = S // P  # 16 t tiles
    NSQ = S // P

    nc = bacc.Bacc(None, target_bir_lowering=False)

    xq_d = nc.declare_dram_parameter("xq", [S // 512, P, 4 * D], bf16, isOutput=False)
    cos_d = nc.declare_dram_parameter("cos2", [P, S], bf16, isOutput=False)
    sin_d = nc.declare_dram_parameter("sin2s", [P, S], bf16, isOutput=False)
    wq_d = nc.declare_dram_parameter("wq", [QH, D, H], bf16, isOutput=False)
    wk_d = nc.declare_dram_parameter("wk", [D, H], bf16, isOutput=False)
    wv_d = nc.declare_dram_parameter("wv", [D, H], bf16, isOutput=False)
    wo_d = nc.declare_dram_parameter("wo", [QH, H, D], bf16, isOutput=False)
    o_d = nc.declare_dram_parameter("o", [S, D], bf16, isOutput=True)

    from contextlib import ExitStack

    with tile.TileContext(nc) as tc, ExitStack() as es:
        # ---------------- pools ----------------
        const = es.enter_context(tc.tile_pool(name="const", bufs=1))
        persist = es.enter_context(tc.tile_pool(name="persist", bufs=1))
        small = es.enter_context(tc.tile_pool(name="small", bufs=2))
        pt_pool = es.enter_context(tc.tile_pool(name="pt", bufs=8))
        ob_pool = es.enter_context(tc.tile_pool(name="ob", bufs=4))
        at_pool = es.enter_context(tc.tile_pool(name="at", bufs=1))
        # PSUM (8 banks): scores ring 2 + shared ring 2 (proj pq/pv,
        # o-proj po, attn transpose ptr2) + 4 AV accumulator banks.
        ps_scr = es.enter_context(tc.tile_pool(name="ps_scr", bufs=2, space="PSUM"))
        ps_pp = es.enter_context(tc.tile_pool(name="ps_pp", bufs=2, space="PSUM"))
        ps_av = es.enter_context(tc.tile_pool(name="ps_av", bufs=1, space="PSUM"))

        # ---------------- constants ----------------
        ident = const.tile([P, P], bf16)
        make_identity(nc, ident)

        exp_bias = const.tile([P, 1], f32)
        nc.gpsimd.memset(exp_bias[:], EXP_BIAS)



        # ---------------- x load -----------------------------------------
        # xT layout [p, st, dk*P + u] : element x(s = st*P + u, d = dk*P + p).
        # The host pre-permutes x into this exact layout, quarter-major, so
        # each st-quarter is ONE fully contiguous DMA (16KB/partition).
        # Quarters 0/1 are split across the sync + scalar queues (scalar is
        # idle until attention starts at ~20us).
        xT = persist.tile([P, NSQ, D], bf16)

        def x_quarter(q, engines):
            n = len(engines)
            hw = RB // n
            for i, eng in enumerate(engines):
                eng.dma_start(
                    xT[:, q * RB + i * hw : q * RB + (i + 1) * hw, :],
                    xq_d[q, :, i * hw * D : (i + 1) * hw * D].rearrange(
                        "p (st d) -> p st d", d=D
                    ),
                )

        x_quarter(0, [nc.sync, nc.scalar])
        # wq1-3 ride the sync queue between x quarters 0 and 1: they are
        # needed at ~20/24/27us, before x q1 (~39us).  Quarters 2/3 are
        # emitted later, behind o-store DMAs, so their transfers don't
        # steal bandwidth from the loads that gate the pipeline start.

        # ---------------- weights (first-use order across queues) ---------
        def load_w_dh(dram_ap, name, eng):  # dram bf16 [D, H] -> sbuf [P, DK, H]
            wb = persist.tile([P, DK, H], bf16, name=name, tag=name)
            eng.dma_start(wb[:], dram_ap.rearrange("(k p) h -> p k h", p=P))
            return wb

        wq_sb = [load_w_dh(wq_d[0], "wq0", nc.gpsimd)]

        # rope tables (host-computed, bf16) right after wq0: first use is
        # the first block's rope-apply at ~15us.
        cos2 = persist.tile([P, S], bf16)
        sin2s = persist.tile([P, S], bf16)
        nc.gpsimd.dma_start(cos2[:], cos_d[:, :])
        nc.gpsimd.dma_start(sin2s[:], sin_d[:, :])

        wq_sb.append(load_w_dh(wq_d[1], "wq1", nc.scalar))
        for h in range(2, QH):
            wq_sb.append(load_w_dh(wq_d[h], f"wq{h}", nc.sync))
        wk_sb = load_w_dh(wk_d, "wk", nc.gpsimd)
        wv_sb = load_w_dh(wv_d, "wv", nc.gpsimd)
        x_quarter(1, [nc.sync])

        # wo tiles declared here; DMAs emitted on gpsimd just before the
        # first attention block (needed at ~40us, keeps them out of the
        # early bandwidth wave).
        wo_sb = []
        for h in range(QH):
            wb = persist.tile([P, D], bf16, name=f"wo{h}", tag=f"wo{h}")
            wo_sb.append(wb)

        def load_wo():
            for h in range(QH):
                nc.gpsimd.dma_start(wo_sb[h][:], wo_d[h])

        # ---------------- q/k projection block (with rope) ----------------
        def proj_qk(w_sb, out_tile, sb):
            pq = ps_pp.tile([P, SB], f32, tag="pp", name="pq")
            for dk in range(DK):
                nc.tensor.matmul(
                    pq[:],
                    w_sb[:, dk, :],
                    xT[:, sb * RB : (sb + 1) * RB, dk * P : (dk + 1) * P],
                    start=(dk == 0),
                    stop=(dk == DK - 1),
                )
            sl = slice(sb * SB, (sb + 1) * SB)
            # rope: out = pq * cos2 + rot(pq) * sin2s
            tsin = small.tile([P, SB], f32, tag="tsin")
            nc.vector.tensor_tensor(
                tsin[0:HH, :], pq[HH:P, :], sin2s[0:HH, sl], MULT
            )
            nc.vector.tensor_tensor(
                tsin[HH:P, :], pq[0:HH, :], sin2s[HH:P, sl], MULT
            )
            tcos = small.tile([P, SB], f32, tag="tcos")
            nc.vector.tensor_tensor(tcos[:], pq[:], cos2[:, sl], MULT)
            nc.vector.tensor_tensor(out_tile[:, sl], tcos[:], tsin[:], ADD)

        qT = [
            persist.tile([P, S], bf16, name=f"qT{h}", tag=f"qT{h}")
            for h in range(QH)
        ]
        kT = persist.tile([P, S], bf16)

        # ---------------- v projection tile (v' with ones column) ---------
        # layout [P, NT, H+4]: v[t = tt*P + p, 0:H], v'[t, H] = 1
        VW = H + 4
        vp = persist.tile([P, NT, VW], bf16)

        def proj_v(tt):
            pv = ps_pp.tile([P, SB], f32, tag="pp", name="pv")[:, 0:P]
            for dk in range(DK):
                nc.tensor.matmul(
                    pv[:],
                    xT[:, tt, dk * P : (dk + 1) * P],
                    wv_sb[:, dk, :],
                    start=(dk == 0),
                    stop=(dk == DK - 1),
                )
            nc.vector.tensor_copy(vp[:, tt, 0:H], pv[:])
            nc.gpsimd.memset(vp[:, tt, H : H + 1], 1.0)

        # ---------------- attention + fused O projection ------------------
        # O-projection matmuls of block sb-1 are fed INTO the attention
        # groups of block sb: each group then carries ~650-900ns of PE work
        # vs the ~570ns scalar-engine exp period, so exp latency stays off
        # the critical path.
        class OprojFeeder:
            """PE filler work fed into attention groups: o-projection tiles
            of the previous block, then v-projection tiles of the next
            quarter (at the tail, after their x quarter has landed)."""

            def __init__(self, attnT_blk, sb, final=False):
                self.units = []  # one unit = one complete po / pv tile
                if attnT_blk is None:
                    self.total = 0
                    self.emitted = 0
                    return
                store_engs = [nc.sync, nc.scalar] if final else [nc.sync]

                def po_tile(st, db):
                    po = ps_pp.tile([P, SB], f32, tag="pp", name="po")
                    for h in range(QH):
                        nc.tensor.matmul(
                            po[:],
                            attnT_blk[h][
                                :, (st - RB * sb) * P : (st - RB * sb + 1) * P
                            ],
                            wo_sb[h][:, db * SB : (db + 1) * SB],
                            start=(h == 0),
                            stop=(h == QH - 1),
                        )
                    ob = ob_pool.tile([P, SB], bf16, tag="ob")
                    nc.vector.tensor_copy(ob[:], po[:])
                    store_engs[(st * 4 + db) % len(store_engs)].dma_start(
                        o_d[st * P : (st + 1) * P, db * SB : (db + 1) * SB],
                        ob[:],
                    )

                for r2 in range(RB):
                    st = RB * sb + r2
                    for db in range(D // SB):
                        self.units.append(
                            lambda st=st, db=db: po_tile(st, db)
                        )
                self.total = len(self.units)
                self.emitted = 0

            def add_units(self, units, front=False):
                if front:
                    self.units = units + self.units
                else:
                    self.units.extend(units)
                self.total += len(units)

            def step(self, group_idx, groups_total):
                # Bresenham pacing: unit k fires at group k*T/N so the
                # feed stays even across the whole block.
                if not self.units:
                    return
                n = self.total
                while self.units and (
                    self.emitted * groups_total <= group_idx * n
                ):
                    self.units.pop(0)()
                    self.emitted += 1

            def drain(self):
                for u in self.units:
                    u()
                self.units = []

        def make_attention_block(sb, attnT, feeder, groups):
            """All heads of block sb as one flat group stream (the LOOK-2
            scores/exp pipeline runs across head boundaries).  Returns
            (warmup, run): warmup emits the first LOOK groups' scores and
            can be called between projection blocks so their exps cook on
            the scalar engine while the PE is still projecting."""
            ntt = RB * (sb + 1)
            LOOK = 2
            seq = [(h, tt) for h in range(QH) for tt in range(ntt)]
            pav_of = {}
            ans = {}
            pts = {}

            def emit_scores(h, tt):
                if (h, tt) in pts:
                    return
                r = tt - RB * sb
                c0 = max(0, r) * P
                pscore = ps_scr.tile([P, SB], f32, tag="scr", name="pscore")
                nc.tensor.matmul(
                    pscore[:, c0:SB],
                    kT[:, tt * P : (tt + 1) * P],
                    qT[h][:, sb * SB + c0 : (sb + 1) * SB],
                    start=True,
                    stop=True,
                )
                pt = pt_pool.tile([P, SB], bf16, tag="pt")
                nc.scalar.activation(
                    pt[:, c0:SB], pscore[:, c0:SB], EXP, bias=exp_bias[:]
                )
                if r >= 0:
                    # causal mask: zero the upper triangle of the diagonal
                    # subtile on gpsimd (idle queue; keeps the DVE out of
                    # the scores->exp->AV chain).
                    nc.gpsimd.affine_select(
                        out=pt[:, c0 : c0 + P],
                        in_=pt[:, c0 : c0 + P],
                        compare_op=mybir.AluOpType.is_ge,
                        fill=0.0,
                        base=0,
                        pattern=[[1, P]],
                        channel_multiplier=-1,
                    )
                pts[(h, tt)] = pt

            def finish_subtile(h, r2):
                rec = small.tile([P, 1], f32, tag="rec", bufs=4)
                nc.vector.reciprocal(rec[:], pav_of[h][r2][:, H : H + 1])
                an = small.tile([P, H], bf16, tag="an", bufs=4)
                nc.vector.tensor_scalar_mul(
                    an[:], pav_of[h][r2][:, 0:H], rec[:]
                )
                ans[(h, r2)] = an

            def emit_transpose(h, r2):
                ptr2 = ps_pp.tile([P, SB], bf16, tag="pp", name="ptr2")[:, 0:P]
                nc.tensor.transpose(ptr2, ans.pop((h, r2))[:], ident[:])
                nc.vector.tensor_copy(
                    attnT[h][:, r2 * P : (r2 + 1) * P], ptr2
                )

            def warmup():
                for i in range(min(LOOK, len(seq))):
                    emit_scores(*seq[i])

            def run():
                warmup()
                feeder.step(groups[0], groups[1])
                groups[0] += 1
                for i, (h, tt) in enumerate(seq):
                    if tt == 0:
                        pav_of[h] = [
                            ps_av.tile(
                                [P, H + 1], f32, name=f"pav{r}",
                                tag=f"av{r}", bufs=1,
                            )[:]
                            for r in range(RB)
                        ]
                    r = tt - RB * sb
                    pt = pts.pop((h, tt))
                    for r2 in range(max(0, r), RB):
                        q128 = RB * sb + r2
                        nc.tensor.matmul(
                            pav_of[h][r2],
                            pt[:, r2 * P : (r2 + 1) * P],
                            vp[:, tt, 0 : H + 1],
                            start=(tt == 0),
                            stop=(tt == q128),
                        )
                    if r >= 0:
                        finish_subtile(h, r)
                    if r >= 1:
                        emit_transpose(h, r - 1)
                    if tt == ntt - 1:
                        emit_transpose(h, RB - 1)
                    feeder.step(groups[0], groups[1])
                    groups[0] += 1
                    if i + LOOK < len(seq):
                        emit_scores(*seq[i + LOOK])

            return warmup, run

        # ---------------- main schedule: per quarter ----------------------
        prev = None
        for sb in range(NSB):
            proj_qk(wq_sb[0], qT[0], sb)
            proj_qk(wk_sb, kT, sb)
            if sb == 0:
                for tt in range(0, RB):
                    proj_v(tt)
            attnT = [
                at_pool.tile(
                    [P, SB], bf16, name=f"attnT{h}", tag=f"attnT{h}", bufs=2
                )
                for h in range(QH)
            ]
            feeder = OprojFeeder(prev, sb - 1)
            if sb + 1 < NSB:
                feeder.add_units(
                    [lambda tt=tt: proj_v(tt)
                     for tt in range((sb + 1) * RB, (sb + 2) * RB)]
                )
            groups_left = [0, QH * RB * (sb + 1) + 1]
            if sb == 0:
                load_wo()
            if sb in (1, 2):
                x_quarter(sb + 1, [nc.sync])
            warmup, run = make_attention_block(sb, attnT, feeder, groups_left)
            # first attention scores between the projection blocks: their
            # exps cook on the scalar engine while the PE projects h1-3.
            warmup()
            for h in range(1, QH):
                proj_qk(wq_sb[h], qT[h], sb)
            run()
            feeder.drain()
            prev = attnT
        OprojFeeder(prev, NSB - 1, final=True).drain()

    nc.compile()
    return nc


_NC_CACHE = {}


def _get_nc(key):
    if key not in _NC_CACHE:
        _NC_CACHE[key] = build_nc(*key)
    return _NC_CACHE[key]


def _rope_tables(positions, H=128, theta=10000.0):
    """cos2/sin2s [P, S] f32 tables matching the device layout:
    cos2[p, s] = cos(ang[s, p mod HH]); sin2s[p, s] = -sin for p < HH,
    +sin for p >= HH."""
    HH = H // 2
    inv_ts = theta ** (-2.0 * np.arange(HH, dtype=np.float64) / H)
    ang = positions.astype(np.float64)[None, :] * inv_ts[:, None]  # [HH, S]
    cos = np.cos(ang).astype(np.float32)
    sin = np.sin(ang).astype(np.float32)
    cos2 = np.concatenate([cos, cos], axis=0).astype(ml_dtypes.bfloat16)
    sin2s = np.concatenate([-sin, sin], axis=0).astype(ml_dtypes.bfloat16)
    return np.ascontiguousarray(cos2), np.ascontiguousarray(sin2s)


def make_in_maps(x, positions, Wq, Wk, Wv, Wo, n_cores=8):
    B, S, D = x.shape
    Q, _, H = Wq.shape
    N = Wk.shape[0]
    groups = Q // N if N else 1
    gpb = n_cores // B  # head groups per batch (4)
    qh_per_core = Q // gpb
    assert qh_per_core * gpb == Q
    scale = np.float32(1.0 / math.sqrt(H))
    cos2, sin2s = _rope_tables(positions, H)
    # xq[q][p, stl, dk, u] = x[(q*4+stl)*128 + u, dk*128 + p]: the exact
    # on-chip xT layout, quarter-major, so each quarter is one plain DMA.
    DKh, RBh, NQh = D // P, 4, S // 512
    xq_b = [
        np.ascontiguousarray(
            x[b]
            .astype(ml_dtypes.bfloat16)
            .reshape(NQh, RBh, P, DKh, P)
            .transpose(0, 4, 1, 3, 2)
            .reshape(NQh, P, RBh * D)
        )
        for b in range(B)
    ]
    in_maps = []
    for c in range(n_cores):
        b = c // gpb
        g = c % gpb
        qh0 = g * qh_per_core
        kvh = qh0 // groups
        in_maps.append(
            {
                "xq": xq_b[b],
                "cos2": cos2,
                "sin2s": sin2s,
                "wq": np.ascontiguousarray(
                    (Wq[qh0 : qh0 + qh_per_core] * scale).astype(ml_dtypes.bfloat16)
                ),
                "wk": np.ascontiguousarray(Wk[kvh].astype(ml_dtypes.bfloat16)),
                "wv": np.ascontiguousarray(Wv[kvh].astype(ml_dtypes.bfloat16)),
                "wo": np.ascontiguousarray(
                    Wo[qh0 : qh0 + qh_per_core].astype(ml_dtypes.bfloat16)
                ),
            }
        )
    return in_maps, gpb, qh_per_core


def kernel(x, positions, Wq, Wk, Wv, Wo):
    """Full inputs -> full output.  x [B,S,D] f32, positions [S] i32,
    Wq [Q,D,H], Wk/Wv [N,D,H], Wo [Q,H,D].  Returns [B,S,D] f32."""
    from concourse.bass_utils import run_bass_kernel_spmd

    x = np.ascontiguousarray(np.asarray(x, dtype=np.float32))
    positions = np.ascontiguousarray(np.asarray(positions, dtype=np.int32))
    Wq = np.asarray(Wq, dtype=np.float32)
    Wk = np.asarray(Wk, dtype=np.float32)
    Wv = np.asarray(Wv, dtype=np.float32)
    Wo = np.asarray(Wo, dtype=np.float32)

    B, S, D = x.shape
    Q, _, H = Wq.shape
    n_cores = 8
    in_maps, gpb, qh_per_core = make_in_maps(x, positions, Wq, Wk, Wv, Wo, n_cores)

    nc = _get_nc((S, D, qh_per_core, H))
    res = run_bass_kernel_spmd(nc, in_maps, core_ids=list(range(n_cores)))
    out = np.zeros((B, S, D), dtype=np.float32)
    for c in range(n_cores):
        out[c // gpb] += res.results[c]["o"].astype(np.float32)
    return out
